# revision 6
# baseline (speedup 1.0000x reference)
"""Multi-head attention (B=4, L=2048, D=1024, H=16) on 8 trn2 NeuronCores.

Sharding: core c -> (batch b = c//2, head-group hg = c%2 of 8 heads).
Each core computes its batch's projections restricted to its 8 heads,
full attention for those (b, h) pairs, returning:
  attn [8, 2048, 2048] f32  and  out [2048, 512] f32
which the host reassembles into the full outputs.

v2 pipeline:
 - Projections in float32r (full PE rate, ~1.6e-4) from host-transposed
   fp32 x / w; PSUM fp32; evacuated to fp16 qT/kT (head-parity partition
   layout: even head on partitions 0-63, odd head on 64-127) and fp16 V in
   natural [l, d] layout with a ones-column per head.
 - Pass A ([k,q] layout): S.T (K=64 fp16) -> exp -> E.T fp16 -> A@V; the
   ones-column makes PSUM row 64 the softmax denominators.
 - Tiny PE transposes of [65,128] O+sums chunks -> DVE reciprocal.
 - Pass B ([q,k] layout): S -> exp -> multiply by 1/sum (per-partition
   scalar, DVE 2x) -> DMA to HBM in the natural attn layout.
"""
import numpy as np
from contextlib import ExitStack

import concourse.bass as bass
import concourse.mybir as mybir
import concourse.tile as tile
from concourse.bass import ds
from concourse.bass_utils import run_bass_kernel_spmd
from concourse.masks import make_identity

AF = mybir.ActivationFunctionType
F32 = mybir.dt.float32
F32R = mybir.dt.float32r
BF16 = mybir.dt.bfloat16
F16 = mybir.dt.float16

B, L, D, H = 4, 2048, 1024, 16
HG = 8          # heads per core
DH = 64         # head dim
P = 128
NCORES = 8
SCALE = 1.0 / 8.0   # 1/sqrt(DH)

_wait_ctr = [0]


def _split_sync_waits(nc, limit=1):
    """walrus (CoreV3) rejects instructions with too many sem waits; hoist
    excess waits onto preceding same-engine NoOps."""
    for f in nc.m.functions:
        for bb in f.blocks:
            out = []
            for inst in bb.instructions:
                si = getattr(inst, "sync_info", None)
                if si is not None and si.on_wait and len(si.on_wait) > limit:
                    waits = list(si.on_wait)
                    keep = waits[-limit:]
                    extra = waits[:-limit]
                    for i in range(0, len(extra), limit):
                        _wait_ctr[0] += 1
                        nop = mybir.InstNoOp(
                            name=f"waitsplit_nop_{_wait_ctr[0]}", ins=[], outs=[]
                        )
                        nop.engine = inst.engine
                        nop.sync_info = mybir.SyncInfo(
                            on_wait=extra[i : i + limit], on_update=[]
                        )
                        out.append(nop)
                    inst.sync_info = mybir.SyncInfo(
                        on_wait=keep, on_update=list(si.on_update)
                    )
                out.append(inst)
            bb.instructions = out
    return nc


def build_nc():
    nc = bass.Bass("TRN2", target_bir_lowering=False, debug=False)

    xs = {
        t: nc.dram_tensor(f"xs_{t}", [D, L], F32, kind="ExternalInput")
        for t in "qkv"
    }
    ws = {
        t: nc.dram_tensor(f"ws_{t}", [D, HG * DH], F32, kind="ExternalInput")
        for t in "qkv"
    }
    attn_o = nc.dram_tensor("attn", [HG, L, L], F32, kind="ExternalOutput")
    out_o = nc.dram_tensor("out", [L, HG * DH], F32, kind="ExternalOutput")

    NLB = L // 512     # 4 l-blocks
    NEO = D // P       # 8 contraction chunks
    NKC = L // P       # 16 k-chunks
    NQC = L // P       # 16 q-chunks

    with tile.TileContext(nc) as tc, ExitStack() as ctx:
        pers = ctx.enter_context(tc.tile_pool(name="pers", bufs=1))
        # head-parity layout: partitions 0-63 = head 2*dc, 64-127 = head 2*dc+1
        qs_sb = pers.tile([P, 4, L], F16, tag="qs")
        ks_sb = pers.tile([P, 4, L], F16, tag="ks")
        v_sb = pers.tile([P, NKC, HG * 65], F16, tag="v")
        out_sb = pers.tile([P, NQC, HG * DH], F32, tag="osb")
        ident = pers.tile([P, P], F32, tag="ident")
        make_identity(nc, ident[:])
        nc.vector.memset(v_sb[:], 1.0)  # ones-columns; data cols overwritten

        # ---------------- Phase 1: projections (float32r) ----------------
        with ExitStack() as c1:
            wpool = c1.enter_context(tc.tile_pool(name="wsb", bufs=1))
            wstg = c1.enter_context(tc.tile_pool(name="wstg", bufs=1))
            xpool = c1.enter_context(tc.tile_pool(name="xsb", bufs=2))
            xrp = c1.enter_context(tc.tile_pool(name="xrp", bufs=1))
            ppsum = c1.enter_context(tc.tile_pool(name="pps", bufs=2, space="PSUM"))

            ws_sb = {}
            for t in "qkv":
                w32 = wstg.tile([P, NEO, HG * DH], F32, tag="wstage")
                nc.sync.dma_start(
                    w32[:], ws[t].ap().rearrange("(eo p) d -> p eo d", p=P)
                )
                ws_sb[t] = wpool.tile(
                    [P, NEO, HG * DH], F32R, tag=f"ws{t}", name=f"ws{t}"
                )
                nc.vector.tensor_copy(ws_sb[t][:], w32[:])

            for t in "qkv":
                dst = qs_sb if t == "q" else (ks_sb if t == "k" else None)
                for lb in range(NLB):
                    x32 = xpool.tile([P, NEO, 512], F32, tag="x32")
                    nc.sync.dma_start(
                        x32[:],
                        xs[t].ap().rearrange("(eo p) l -> p eo l", p=P)[
                            :, :, ds(lb * 512, 512)
                        ],
                    )
                    xrt = xrp.tile([P, NEO, 512], F32R, tag="xr")
                    nc.vector.tensor_copy(xrt[:], x32[:])
                    xr = xrt[:]
                    if t in "qk":
                        # transposed layout: psum [128 dchunk, 512 l]
                        for dc in range(4):
                            pst = ppsum.tile([P, 512], F32, tag="ps")
                            for eo in range(NEO):
                                nc.tensor.matmul(
                                    pst[:],
                                    ws_sb[t][:, eo, ds(dc * P, P)],
                                    xr[:, eo, :],
                                    start=(eo == 0),
                                    stop=(eo == NEO - 1),
                                )
                            # head-parity aligned: just one fp16 cast copy
                            nc.vector.tensor_copy(
                                dst[:, dc, ds(lb * 512, 512)], pst[:]
                            )
                    else:
                        # natural layout: psum [128 l, 512 d]
                        for lc2 in range(4):
                            pst = ppsum.tile([P, 512], F32, tag="ps")
                            for eo in range(NEO):
                                nc.tensor.matmul(
                                    pst[:],
                                    xr[:, eo, ds(lc2 * P, P)],
                                    ws_sb["v"][:, eo, :],
                                    start=(eo == 0),
                                    stop=(eo == NEO - 1),
                                )
                            lc = lb * 4 + lc2
                            vv = v_sb[:, lc].rearrange("p (h x) -> p h x", h=HG)
                            nc.vector.tensor_copy(
                                vv[:, :, 0:64],
                                pst[:].rearrange("p (h d) -> p h d", h=HG),
                            )

        # ---------------- Phase 2: attention (head pairs) ----------------
        etp = ctx.enter_context(tc.tile_pool(name="et", bufs=4))
        e2p = ctx.enter_context(tc.tile_pool(name="e2", bufs=4))
        pstg = ctx.enter_context(tc.tile_pool(name="pst", bufs=3))
        oaugp = ctx.enter_context(tc.tile_pool(name="oaug", bufs=2))
        rp = ctx.enter_context(tc.tile_pool(name="recip", bufs=2))
        # st pool: [128,1024] 2-bank slots, also hosts pass-B sb and tr tiles
        st_ps = ctx.enter_context(tc.tile_pool(name="stp", bufs=2, space="PSUM"))
        av_ps = ctx.enter_context(tc.tile_pool(name="avp", bufs=2, space="PSUM"))

        for dc in range(4):  # head pairs: heads 2dc (parts 0-63), 2dc+1 (64-127)
            recs = [rp.tile([P, NQC], F32, tag="rec", name=f"rec{i}") for i in range(2)]
            for qbp in range(2):  # 1024 q at a time
                avs = [
                    av_ps.tile([65, 1024], F32, tag="av", name=f"av{i}")
                    for i in range(2)
                ]
                for kc in range(NKC):
                    sts = [
                        st_ps.tile([P, 1024], F32, tag="st", name=f"st{i}")
                        for i in range(2)
                    ]
                    for qh in range(2):
                        q0 = qbp * 1024 + qh * 512
                        for i in range(2):
                            off = i * 64
                            nc.tensor.matmul(
                                sts[i][:, ds(qh * 512, 512)],
                                ks_sb[ds(off, 64), dc, ds(kc * P, P)],
                                qs_sb[ds(off, 64), dc, ds(q0, 512)],
                                start=True,
                                stop=True,
                            )
                    ets = []
                    for i in range(2):
                        et = etp.tile([P, 1024], F16, tag="et", name=f"et{i}")
                        nc.scalar.activation(et[:], sts[i][:], AF.Exp, scale=SCALE)
                        ets.append(et)
                    for qh in range(2):
                        for i in range(2):
                            h = 2 * dc + i
                            nc.tensor.matmul(
                                avs[i][:, ds(qh * 512, 512)],
                                v_sb[:, kc, ds(h * 65, 65)],
                                ets[i][:, ds(qh * 512, 512)],
                                start=(kc == 0),
                                stop=(kc == NKC - 1),
                            )
                for i in range(2):
                    h = 2 * dc + i
                    oaug = oaugp.tile([65, 1024], F32, tag="oaug", name=f"oaug{i}")
                    nc.vector.tensor_copy(oaug[:], avs[i][:])
                    for half in range(2):
                        tr = st_ps.tile([P, 4, 65], F32, tag="st", name="tr")
                        for j in range(4):
                            chunk = half * 4 + j
                            nc.tensor.transpose(
                                tr[:, j, :],
                                oaug[:, ds(chunk * P, P)],
                                ident[0:65, 0:65],
                            )
                        qc0 = qbp * 8 + half * 4
                        nc.vector.reciprocal(recs[i][:, ds(qc0, 4)], tr[:, :, 64])
                        for j in range(4):
                            qc = qc0 + j
                            nc.vector.tensor_scalar_mul(
                                out_sb[:, qc, ds(h * DH, DH)],
                                tr[:, j, 0:DH],
                                recs[i][:, ds(qc, 1)],
                            )
            # pass B: natural-layout S -> normalized attention weights
            for qc in range(NQC):
                psts = [
                    pstg.tile([P, L], F32, tag="pstage", name=f"pst{i}")
                    for i in range(2)
                ]
                for kh in range(2):
                    sbs = [
                        st_ps.tile([P, 1024], F32, tag="st", name=f"sb{i}")
                        for i in range(2)
                    ]
                    for kb2 in range(2):
                        for i in range(2):
                            off = i * 64
                            nc.tensor.matmul(
                                sbs[i][:, ds(kb2 * 512, 512)],
                                qs_sb[ds(off, 64), dc, ds(qc * P, P)],
                                ks_sb[ds(off, 64), dc, ds(kh * 1024 + kb2 * 512, 512)],
                                start=True,
                                stop=True,
                            )
                    for i in range(2):
                        e2 = e2p.tile([P, 1024], F32, tag="e2", name=f"e2{i}")
                        nc.scalar.activation(e2[:], sbs[i][:], AF.Exp, scale=SCALE)
                        nc.vector.tensor_scalar_mul(
                            psts[i][:, ds(kh * 1024, 1024)],
                            e2[:],
                            recs[i][:, ds(qc, 1)],
                        )
                for i in range(2):
                    h = 2 * dc + i
                    nc.sync.dma_start(
                        attn_o.ap()[h, ds(qc * P, P), :], psts[i][:]
                    )

        for qc in range(NQC):
            nc.sync.dma_start(out_o.ap()[ds(qc * P, P), :], out_sb[:, qc, :])

    return _split_sync_waits(nc)


_NC = None


def _get_nc():
    global _NC
    if _NC is None:
        _NC = build_nc()
    return _NC


def _prep_in_maps(query, key, value, w_q, w_k, w_v):
    x = {
        "q": np.asarray(query, dtype=np.float32),
        "k": np.asarray(key, dtype=np.float32),
        "v": np.asarray(value, dtype=np.float32),
    }
    w = {
        "q": np.asarray(w_q, dtype=np.float32),
        "k": np.asarray(w_k, dtype=np.float32),
        "v": np.asarray(w_v, dtype=np.float32),
    }
    ws_c = {
        t: [
            np.ascontiguousarray(w[t][hg * HG * DH : (hg + 1) * HG * DH, :].T)
            for hg in range(2)
        ]
        for t in "qkv"
    }
    in_maps = []
    for c in range(NCORES):
        b, hg = divmod(c, 2)
        m = {}
        for t in "qkv":
            m[f"xs_{t}"] = np.ascontiguousarray(x[t][b].T)
            m[f"ws_{t}"] = ws_c[t][hg]
        in_maps.append(m)
    return in_maps


def _assemble(results):
    out = np.empty((B, L, D), np.float32)
    attn = np.empty((B, H, L, L), np.float32)
    for c in range(NCORES):
        b, hg = divmod(c, 2)
        attn[b, hg * HG : (hg + 1) * HG] = results[c]["attn"]
        out[b, :, hg * HG * DH : (hg + 1) * HG * DH] = results[c]["out"]
    return out, attn


def run(in_maps, trace=False, **kwargs):
    nc = _get_nc()
    return run_bass_kernel_spmd(
        nc, in_maps, core_ids=list(range(NCORES)), trace=trace, **kwargs
    )


def kernel(query, key, value, w_q, w_k, w_v):
    in_maps = _prep_in_maps(query, key, value, w_q, w_k, w_v)
    res = run(in_maps)
    return _assemble(res.results)


# revision 10
# speedup vs baseline: 1.3000x; 1.3000x over previous
"""Multi-head attention (B=4, L=2048, D=1024, H=16) on 8 trn2 NeuronCores.

Sharding: core c -> (batch b = c//2, head-group hg = c%2 of 8 heads).
Each core computes its batch's projections restricted to its 8 heads,
full attention for those (b, h) pairs, returning:
  attn [8, 2048, 2048] f32  and  out [2048, 512] f32
which the host reassembles into the full outputs.

v2 pipeline:
 - Projections in float32r (full PE rate, ~1.6e-4) from host-transposed
   fp32 x / w; PSUM fp32; evacuated to fp16 qT/kT (head-parity partition
   layout: even head on partitions 0-63, odd head on 64-127) and fp16 V in
   natural [l, d] layout with a ones-column per head.
 - Pass A ([k,q] layout): S.T (K=64 fp16) -> exp -> E.T fp16 -> A@V; the
   ones-column makes PSUM row 64 the softmax denominators.
 - Tiny PE transposes of [65,128] O+sums chunks -> DVE reciprocal.
 - Pass B ([q,k] layout): S -> exp -> multiply by 1/sum (per-partition
   scalar, DVE 2x) -> DMA to HBM in the natural attn layout.
"""
import numpy as np
from contextlib import ExitStack

import concourse.bass as bass
import concourse.mybir as mybir
import concourse.tile as tile
from concourse.bass import ds
from concourse.bass_utils import run_bass_kernel_spmd
from concourse.masks import make_identity

AF = mybir.ActivationFunctionType
F32 = mybir.dt.float32
F32R = mybir.dt.float32r
BF16 = mybir.dt.bfloat16
F16 = mybir.dt.float16

B, L, D, H = 4, 2048, 1024, 16
HG = 8          # heads per core
DH = 64         # head dim
P = 128
NCORES = 8
SCALE = 1.0 / 8.0   # 1/sqrt(DH)

_wait_ctr = [0]


def _split_sync_waits(nc, limit=1):
    """walrus (CoreV3) rejects instructions with too many sem waits; hoist
    excess waits onto preceding same-engine NoOps."""
    for f in nc.m.functions:
        for bb in f.blocks:
            out = []
            for inst in bb.instructions:
                si = getattr(inst, "sync_info", None)
                if si is not None and si.on_wait and len(si.on_wait) > limit:
                    waits = list(si.on_wait)
                    keep = waits[-limit:]
                    extra = waits[:-limit]
                    for i in range(0, len(extra), limit):
                        _wait_ctr[0] += 1
                        nop = mybir.InstNoOp(
                            name=f"waitsplit_nop_{_wait_ctr[0]}", ins=[], outs=[]
                        )
                        nop.engine = inst.engine
                        nop.sync_info = mybir.SyncInfo(
                            on_wait=extra[i : i + limit], on_update=[]
                        )
                        out.append(nop)
                    inst.sync_info = mybir.SyncInfo(
                        on_wait=keep, on_update=list(si.on_update)
                    )
                out.append(inst)
            bb.instructions = out
    return nc


def build_nc():
    nc = bass.Bass("TRN2", target_bir_lowering=False, debug=False)

    xs = {
        t: nc.dram_tensor(f"xs_{t}", [D, L], F32, kind="ExternalInput")
        for t in "qkv"
    }
    ws = {
        t: nc.dram_tensor(f"ws_{t}", [D, HG * DH], F32, kind="ExternalInput")
        for t in "qkv"
    }
    attn_o = nc.dram_tensor("attn", [HG, L, L], F32, kind="ExternalOutput")
    out_o = nc.dram_tensor("out", [L, HG * DH], F32, kind="ExternalOutput")

    NLB = L // 512     # 4 l-blocks
    NEO = D // P       # 8 contraction chunks
    NKC = L // P       # 16 k-chunks
    NQC = L // P       # 16 q-chunks

    with tile.TileContext(nc) as tc, ExitStack() as ctx:
        pers = ctx.enter_context(tc.tile_pool(name="pers", bufs=1))
        # per-head K-padded layout: partitions 0-63 = head dh rows, 64-127 = 0
        qs_sb = pers.tile([P, HG, L], F16, tag="qs")
        ks_sb = pers.tile([P, HG, L], F16, tag="ks")
        v_sb = pers.tile([P, NKC, HG * 65], F16, tag="v")
        out_sb = pers.tile([P, NQC, HG * DH], F32, tag="osb")
        ident = pers.tile([P, P], F32, tag="ident")
        make_identity(nc, ident[:])
        nc.vector.memset(v_sb[:], 1.0)  # ones-columns; data cols overwritten
        nc.vector.memset(qs_sb[ds(64, 64)], 0.0)  # zero K-pad halves
        nc.vector.memset(ks_sb[ds(64, 64)], 0.0)

        # ---------------- Phase 1: projections (float32r) ----------------
        with ExitStack() as c1:
            wpool = c1.enter_context(tc.tile_pool(name="wsb", bufs=1))
            wstg = c1.enter_context(tc.tile_pool(name="wstg", bufs=1))
            xpool = c1.enter_context(tc.tile_pool(name="xsb", bufs=1))
            xrp = c1.enter_context(tc.tile_pool(name="xrp", bufs=1))
            hstg = c1.enter_context(tc.tile_pool(name="hstg", bufs=3))
            ppsum = c1.enter_context(tc.tile_pool(name="pps", bufs=2, space="PSUM"))

            ws_sb = {}
            for t in "qkv":
                ws_sb[t] = wpool.tile(
                    [P, NEO, HG * DH], F32R, tag=f"ws{t}", name=f"ws{t}"
                )
                for wh in range(2):
                    w32 = wstg.tile([P, NEO // 2, HG * DH], F32, tag="wstage")
                    nc.sync.dma_start(
                        w32[:],
                        ws[t].ap().rearrange("(eo p) d -> p eo d", p=P)[
                            :, ds(wh * (NEO // 2), NEO // 2), :
                        ],
                    )
                    nc.vector.tensor_copy(
                        ws_sb[t][:, ds(wh * (NEO // 2), NEO // 2), :], w32[:]
                    )

            for t in "qkv":
                dst = qs_sb if t == "q" else (ks_sb if t == "k" else None)
                for lb in range(NLB):
                    x32 = xpool.tile([P, NEO, 512], F32, tag="x32")
                    nc.sync.dma_start(
                        x32[:],
                        xs[t].ap().rearrange("(eo p) l -> p eo l", p=P)[
                            :, :, ds(lb * 512, 512)
                        ],
                    )
                    xrt = xrp.tile([P, NEO, 512], F32R, tag="xr")
                    nc.vector.tensor_copy(xrt[:], x32[:])
                    xr = xrt[:]
                    if t in "qk":
                        # transposed layout: psum [128 dchunk, 512 l]
                        for dc in range(4):
                            pst = ppsum.tile([P, 512], F32, tag="ps")
                            for eo in range(NEO):
                                nc.tensor.matmul(
                                    pst[:],
                                    ws_sb[t][:, eo, ds(dc * P, P)],
                                    xr[:, eo, :],
                                    start=(eo == 0),
                                    stop=(eo == NEO - 1),
                                )
                            # even head: partitions 0-63 aligned, direct copy
                            nc.vector.tensor_copy(
                                dst[0:64, 2 * dc, ds(lb * 512, 512)],
                                pst[0:64, :],
                            )
                            # odd head: stage fp16 then partition-shift via DMA
                            stg = hstg.tile([P, 512], F16, tag="hstg")
                            nc.vector.tensor_copy(stg[64:128, :], pst[64:128, :])
                            nc.sync.dma_start(
                                dst[0:64, 2 * dc + 1, ds(lb * 512, 512)],
                                stg[64:128, :],
                            )
                    else:
                        # natural layout: psum [128 l, 512 d]
                        for lc2 in range(4):
                            pst = ppsum.tile([P, 512], F32, tag="ps")
                            for eo in range(NEO):
                                nc.tensor.matmul(
                                    pst[:],
                                    xr[:, eo, ds(lc2 * P, P)],
                                    ws_sb["v"][:, eo, :],
                                    start=(eo == 0),
                                    stop=(eo == NEO - 1),
                                )
                            lc = lb * 4 + lc2
                            vv = v_sb[:, lc].rearrange("p (h x) -> p h x", h=HG)
                            nc.vector.tensor_copy(
                                vv[:, :, 0:64],
                                pst[:].rearrange("p (h d) -> p h d", h=HG),
                            )

        # ---------------- Phase 2: attention ----------------
        etp = ctx.enter_context(tc.tile_pool(name="et", bufs=4))
        e2p = ctx.enter_context(tc.tile_pool(name="e2", bufs=4))
        pstg = ctx.enter_context(tc.tile_pool(name="pst", bufs=3))
        oaugp = ctx.enter_context(tc.tile_pool(name="oaug", bufs=2))
        rp = ctx.enter_context(tc.tile_pool(name="recip", bufs=2))
        # st pool [128,1024] 2-bank slots; pass-B sb and tr tiles share it
        st_ps = ctx.enter_context(tc.tile_pool(name="stp", bufs=2, space="PSUM"))
        av_ps = ctx.enter_context(tc.tile_pool(name="avp", bufs=2, space="PSUM"))

        for h in range(HG):
            rec = rp.tile([P, NQC], F32, tag="rec")
            for qbp in range(2):  # 1024 q at a time
                av = av_ps.tile([65, 1024], F32, tag="av", name="av")
                for kc in range(NKC):
                    st = st_ps.tile([P, 1024], F32, tag="st", name="st")
                    for qh in range(2):
                        q0 = qbp * 1024 + qh * 512
                        nc.tensor.matmul(
                            st[:, ds(qh * 512, 512)],
                            ks_sb[:, h, ds(kc * P, P)],
                            qs_sb[:, h, ds(q0, 512)],
                            start=True,
                            stop=True,
                        )
                    et = etp.tile([P, 1024], F16, tag="et")
                    nc.scalar.activation(et[:], st[:], AF.Exp, scale=SCALE)
                    for qh in range(2):
                        nc.tensor.matmul(
                            av[:, ds(qh * 512, 512)],
                            v_sb[:, kc, ds(h * 65, 65)],
                            et[:, ds(qh * 512, 512)],
                            start=(kc == 0),
                            stop=(kc == NKC - 1),
                        )
                oaug = oaugp.tile([65, 1024], F32, tag="oaug")
                nc.vector.tensor_copy(oaug[:], av[:])
                for half in range(2):
                    tr = st_ps.tile([P, 4, 65], F32, tag="st", name="tr")
                    for j in range(4):
                        chunk = half * 4 + j
                        nc.tensor.transpose(
                            tr[:, j, :],
                            oaug[:, ds(chunk * P, P)],
                            ident[0:65, 0:65],
                        )
                    qc0 = qbp * 8 + half * 4
                    nc.vector.reciprocal(rec[:, ds(qc0, 4)], tr[:, :, 64])
                    for j in range(4):
                        qc = qc0 + j
                        nc.vector.tensor_scalar_mul(
                            out_sb[:, qc, ds(h * DH, DH)],
                            tr[:, j, 0:DH],
                            rec[:, ds(qc, 1)],
                        )
            # pass B: natural-layout S -> normalized attention weights
            for qc in range(NQC):
                pstage = pstg.tile([P, L], F32, tag="pstage")
                for kh in range(2):
                    sb = st_ps.tile([P, 1024], F32, tag="st", name="sb")
                    for kb2 in range(2):
                        nc.tensor.matmul(
                            sb[:, ds(kb2 * 512, 512)],
                            qs_sb[:, h, ds(qc * P, P)],
                            ks_sb[:, h, ds(kh * 1024 + kb2 * 512, 512)],
                            start=True,
                            stop=True,
                        )
                    e2 = e2p.tile([P, 1024], F32, tag="e2")
                    nc.scalar.activation(e2[:], sb[:], AF.Exp, scale=SCALE)
                    nc.vector.tensor_scalar_mul(
                        pstage[:, ds(kh * 1024, 1024)], e2[:], rec[:, ds(qc, 1)]
                    )
                nc.sync.dma_start(attn_o.ap()[h, ds(qc * P, P), :], pstage[:])

        for qc in range(NQC):
            nc.sync.dma_start(out_o.ap()[ds(qc * P, P), :], out_sb[:, qc, :])

    return _split_sync_waits(nc)


_NC = None


def _get_nc():
    global _NC
    if _NC is None:
        _NC = build_nc()
    return _NC


def _prep_in_maps(query, key, value, w_q, w_k, w_v):
    x = {
        "q": np.asarray(query, dtype=np.float32),
        "k": np.asarray(key, dtype=np.float32),
        "v": np.asarray(value, dtype=np.float32),
    }
    w = {
        "q": np.asarray(w_q, dtype=np.float32),
        "k": np.asarray(w_k, dtype=np.float32),
        "v": np.asarray(w_v, dtype=np.float32),
    }
    ws_c = {
        t: [
            np.ascontiguousarray(w[t][hg * HG * DH : (hg + 1) * HG * DH, :].T)
            for hg in range(2)
        ]
        for t in "qkv"
    }
    in_maps = []
    for c in range(NCORES):
        b, hg = divmod(c, 2)
        m = {}
        for t in "qkv":
            m[f"xs_{t}"] = np.ascontiguousarray(x[t][b].T)
            m[f"ws_{t}"] = ws_c[t][hg]
        in_maps.append(m)
    return in_maps


def _assemble(results):
    out = np.empty((B, L, D), np.float32)
    attn = np.empty((B, H, L, L), np.float32)
    for c in range(NCORES):
        b, hg = divmod(c, 2)
        attn[b, hg * HG : (hg + 1) * HG] = results[c]["attn"]
        out[b, :, hg * HG * DH : (hg + 1) * HG * DH] = results[c]["out"]
    return out, attn


def run(in_maps, trace=False, **kwargs):
    nc = _get_nc()
    return run_bass_kernel_spmd(
        nc, in_maps, core_ids=list(range(NCORES)), trace=trace, **kwargs
    )


def kernel(query, key, value, w_q, w_k, w_v):
    in_maps = _prep_in_maps(query, key, value, w_q, w_k, w_v)
    res = run(in_maps)
    return _assemble(res.results)


# revision 11
# speedup vs baseline: 1.4915x; 1.1473x over previous
"""Multi-head attention (B=4, L=2048, D=1024, H=16) on 8 trn2 NeuronCores.

Sharding: core c -> (batch b = c//2, head-group hg = c%2 of 8 heads).
Each core computes its batch's projections restricted to its 8 heads,
full attention for those (b, h) pairs, returning:
  attn [8, 2048, 2048] f32  and  out [2048, 512] f32
which the host reassembles into the full outputs.

v2 pipeline:
 - Projections in float32r (full PE rate, ~1.6e-4) from host-transposed
   fp32 x / w; PSUM fp32; evacuated to fp16 qT/kT (head-parity partition
   layout: even head on partitions 0-63, odd head on 64-127) and fp16 V in
   natural [l, d] layout with a ones-column per head.
 - Pass A ([k,q] layout): S.T (K=64 fp16) -> exp -> E.T fp16 -> A@V; the
   ones-column makes PSUM row 64 the softmax denominators.
 - Tiny PE transposes of [65,128] O+sums chunks -> DVE reciprocal.
 - Pass B ([q,k] layout): S -> exp -> multiply by 1/sum (per-partition
   scalar, DVE 2x) -> DMA to HBM in the natural attn layout.
"""
import numpy as np
from contextlib import ExitStack

import concourse.bass as bass
import concourse.mybir as mybir
import concourse.tile as tile
from concourse.bass import ds
from concourse.bass_utils import run_bass_kernel_spmd
from concourse.masks import make_identity

AF = mybir.ActivationFunctionType
F32 = mybir.dt.float32
F32R = mybir.dt.float32r
BF16 = mybir.dt.bfloat16
F16 = mybir.dt.float16

B, L, D, H = 4, 2048, 1024, 16
HG = 8          # heads per core
DH = 64         # head dim
P = 128
NCORES = 8
SCALE = 1.0 / 8.0   # 1/sqrt(DH)

_wait_ctr = [0]


def _split_sync_waits(nc, limit=1):
    """walrus (CoreV3) rejects instructions with too many sem waits; hoist
    excess waits onto preceding same-engine NoOps."""
    for f in nc.m.functions:
        for bb in f.blocks:
            out = []
            for inst in bb.instructions:
                si = getattr(inst, "sync_info", None)
                if si is not None and si.on_wait and len(si.on_wait) > limit:
                    waits = list(si.on_wait)
                    keep = waits[-limit:]
                    extra = waits[:-limit]
                    for i in range(0, len(extra), limit):
                        _wait_ctr[0] += 1
                        nop = mybir.InstNoOp(
                            name=f"waitsplit_nop_{_wait_ctr[0]}", ins=[], outs=[]
                        )
                        nop.engine = inst.engine
                        nop.sync_info = mybir.SyncInfo(
                            on_wait=extra[i : i + limit], on_update=[]
                        )
                        out.append(nop)
                    inst.sync_info = mybir.SyncInfo(
                        on_wait=keep, on_update=list(si.on_update)
                    )
                out.append(inst)
            bb.instructions = out
    return nc


def build_nc():
    nc = bass.Bass("TRN2", target_bir_lowering=False, debug=False)

    xs = {
        t: nc.dram_tensor(f"xs_{t}", [D, L], F32, kind="ExternalInput")
        for t in "qkv"
    }
    ws = {
        t: nc.dram_tensor(f"ws_{t}", [D, HG * DH], F32, kind="ExternalInput")
        for t in "qkv"
    }
    attn_o = nc.dram_tensor("attn", [HG, L, L], F32, kind="ExternalOutput")
    out_o = nc.dram_tensor("out", [L, HG * DH], F32, kind="ExternalOutput")

    NLB = L // 512     # 4 l-blocks
    NEO = D // P       # 8 contraction chunks
    NKC = L // P       # 16 k-chunks
    NQC = L // P       # 16 q-chunks

    with tile.TileContext(nc) as tc, ExitStack() as ctx:
        pers = ctx.enter_context(tc.tile_pool(name="pers", bufs=1))
        # per-head K-padded layout: partitions 0-63 = head dh rows, 64-127 = 0
        qs_sb = pers.tile([P, HG, L], F16, tag="qs")
        ks_sb = pers.tile([P, HG, L], F16, tag="ks")
        v_sb = pers.tile([P, NKC, HG * 65], F16, tag="v")
        ident = pers.tile([P, P], F32, tag="ident")
        make_identity(nc, ident[:])
        nc.vector.memset(v_sb[:], 1.0)  # ones-columns; data cols overwritten
        nc.vector.memset(qs_sb[ds(64, 64)], 0.0)  # zero K-pad halves
        nc.vector.memset(ks_sb[ds(64, 64)], 0.0)

        # ---------------- Phase 1: projections (float32r) ----------------
        with ExitStack() as c1:
            wpool = c1.enter_context(tc.tile_pool(name="wsb", bufs=1))
            wstg = c1.enter_context(tc.tile_pool(name="wstg", bufs=1))
            xpool = c1.enter_context(tc.tile_pool(name="xsb", bufs=2))
            xrp = c1.enter_context(tc.tile_pool(name="xrp", bufs=2))
            hstg = c1.enter_context(tc.tile_pool(name="hstg", bufs=3))
            ppsum = c1.enter_context(tc.tile_pool(name="pps", bufs=2, space="PSUM"))

            ws_sb = {}
            for t in "qkv":
                ws_sb[t] = wpool.tile(
                    [P, NEO, HG * DH], F32R, tag=f"ws{t}", name=f"ws{t}"
                )
                for wh in range(2):
                    w32 = wstg.tile([P, NEO // 2, HG * DH], F32, tag="wstage")
                    nc.sync.dma_start(
                        w32[:],
                        ws[t].ap().rearrange("(eo p) d -> p eo d", p=P)[
                            :, ds(wh * (NEO // 2), NEO // 2), :
                        ],
                    )
                    nc.vector.tensor_copy(
                        ws_sb[t][:, ds(wh * (NEO // 2), NEO // 2), :], w32[:]
                    )

            for t in "qkv":
                dst = qs_sb if t == "q" else (ks_sb if t == "k" else None)
                for lb in range(NLB):
                    x32 = xpool.tile([P, NEO, 512], F32, tag="x32")
                    nc.sync.dma_start(
                        x32[:],
                        xs[t].ap().rearrange("(eo p) l -> p eo l", p=P)[
                            :, :, ds(lb * 512, 512)
                        ],
                    )
                    xrt = xrp.tile([P, NEO, 512], F32R, tag="xr")
                    nc.vector.tensor_copy(xrt[:], x32[:])
                    xr = xrt[:]
                    if t in "qk":
                        # transposed layout: psum [128 dchunk, 512 l]
                        for dc in range(4):
                            pst = ppsum.tile([P, 512], F32, tag="ps")
                            for eo in range(NEO):
                                nc.tensor.matmul(
                                    pst[:],
                                    ws_sb[t][:, eo, ds(dc * P, P)],
                                    xr[:, eo, :],
                                    start=(eo == 0),
                                    stop=(eo == NEO - 1),
                                )
                            # even head: partitions 0-63 aligned, direct copy
                            nc.vector.tensor_copy(
                                dst[0:64, 2 * dc, ds(lb * 512, 512)],
                                pst[0:64, :],
                            )
                            # odd head: stage fp16 then partition-shift via DMA
                            stg = hstg.tile([P, 512], F16, tag="hstg")
                            nc.vector.tensor_copy(stg[64:128, :], pst[64:128, :])
                            nc.sync.dma_start(
                                dst[0:64, 2 * dc + 1, ds(lb * 512, 512)],
                                stg[64:128, :],
                            )
                    else:
                        # natural layout: psum [128 l, 512 d]
                        for lc2 in range(4):
                            pst = ppsum.tile([P, 512], F32, tag="ps")
                            for eo in range(NEO):
                                nc.tensor.matmul(
                                    pst[:],
                                    xr[:, eo, ds(lc2 * P, P)],
                                    ws_sb["v"][:, eo, :],
                                    start=(eo == 0),
                                    stop=(eo == NEO - 1),
                                )
                            lc = lb * 4 + lc2
                            vv = v_sb[:, lc].rearrange("p (h x) -> p h x", h=HG)
                            nc.vector.tensor_copy(
                                vv[:, :, 0:64],
                                pst[:].rearrange("p (h d) -> p h d", h=HG),
                            )

        # ---------------- Phase 2: attention ----------------
        etp = ctx.enter_context(tc.tile_pool(name="et", bufs=4))
        e2p = ctx.enter_context(tc.tile_pool(name="e2", bufs=4))
        pstg = ctx.enter_context(tc.tile_pool(name="pst", bufs=3))
        oaugp = ctx.enter_context(tc.tile_pool(name="oaug", bufs=2))
        ostg = ctx.enter_context(tc.tile_pool(name="ostg", bufs=4))
        rp = ctx.enter_context(tc.tile_pool(name="recip", bufs=2))
        # st pool [128,1024] 2-bank slots; pass-B sb tiles share it
        st_ps = ctx.enter_context(tc.tile_pool(name="stp", bufs=2, space="PSUM"))
        # av pool [65,1024] 2-bank slots; tr tiles share it
        av_ps = ctx.enter_context(tc.tile_pool(name="avp", bufs=2, space="PSUM"))

        for h in range(HG):
            rec = rp.tile([P, NQC], F32, tag="rec")
            for qbp in range(2):  # 1024 q at a time
                av = av_ps.tile([65, 1024], F32, tag="av", name="av")
                for kc in range(NKC):
                    st = st_ps.tile([P, 1024], F32, tag="st", name="st")
                    for qh in range(2):
                        q0 = qbp * 1024 + qh * 512
                        nc.tensor.matmul(
                            st[:, ds(qh * 512, 512)],
                            ks_sb[:, h, ds(kc * P, P)],
                            qs_sb[:, h, ds(q0, 512)],
                            start=True,
                            stop=True,
                        )
                    et = etp.tile([P, 1024], F16, tag="et")
                    nc.scalar.activation(et[:], st[:], AF.Exp, scale=SCALE)
                    for qh in range(2):
                        nc.tensor.matmul(
                            av[:, ds(qh * 512, 512)],
                            v_sb[:, kc, ds(h * 65, 65)],
                            et[:, ds(qh * 512, 512)],
                            start=(kc == 0),
                            stop=(kc == NKC - 1),
                        )
                oaug = oaugp.tile([65, 1024], F32, tag="oaug")
                nc.vector.tensor_copy(oaug[:], av[:])
                for half in range(2):
                    tr = av_ps.tile([P, 4, 65], F32, tag="av", name="tr")
                    for j in range(4):
                        chunk = half * 4 + j
                        nc.tensor.transpose(
                            tr[:, j, :],
                            oaug[:, ds(chunk * P, P)],
                            ident[0:65, 0:65],
                        )
                    qc0 = qbp * 8 + half * 4
                    nc.vector.reciprocal(rec[:, ds(qc0, 4)], tr[:, :, 64])
                    for j in range(4):
                        qc = qc0 + j
                        og = ostg.tile([P, DH], F32, tag="og")
                        nc.vector.tensor_scalar_mul(
                            og[:], tr[:, j, 0:DH], rec[:, ds(qc, 1)]
                        )
                        nc.sync.dma_start(
                            out_o.ap()[ds(qc * P, P), ds(h * DH, DH)], og[:]
                        )
                # pass B for this qbp's q-chunks (overlaps next kc-loop)
                for qc in range(qbp * 8, qbp * 8 + 8):
                    pstage = pstg.tile([P, L], F32, tag="pstage")
                    for kh in range(2):
                        sb = st_ps.tile([P, 1024], F32, tag="st", name="sb")
                        for kb2 in range(2):
                            nc.tensor.matmul(
                                sb[:, ds(kb2 * 512, 512)],
                                qs_sb[:, h, ds(qc * P, P)],
                                ks_sb[:, h, ds(kh * 1024 + kb2 * 512, 512)],
                                start=True,
                                stop=True,
                            )
                        e2 = e2p.tile([P, 1024], F32, tag="e2")
                        nc.scalar.activation(e2[:], sb[:], AF.Exp, scale=SCALE)
                        nc.vector.tensor_scalar_mul(
                            pstage[:, ds(kh * 1024, 1024)], e2[:], rec[:, ds(qc, 1)]
                        )
                    nc.sync.dma_start(attn_o.ap()[h, ds(qc * P, P), :], pstage[:])

    return _split_sync_waits(nc)


_NC = None


def _get_nc():
    global _NC
    if _NC is None:
        _NC = build_nc()
    return _NC


def _prep_in_maps(query, key, value, w_q, w_k, w_v):
    x = {
        "q": np.asarray(query, dtype=np.float32),
        "k": np.asarray(key, dtype=np.float32),
        "v": np.asarray(value, dtype=np.float32),
    }
    w = {
        "q": np.asarray(w_q, dtype=np.float32),
        "k": np.asarray(w_k, dtype=np.float32),
        "v": np.asarray(w_v, dtype=np.float32),
    }
    ws_c = {
        t: [
            np.ascontiguousarray(w[t][hg * HG * DH : (hg + 1) * HG * DH, :].T)
            for hg in range(2)
        ]
        for t in "qkv"
    }
    in_maps = []
    for c in range(NCORES):
        b, hg = divmod(c, 2)
        m = {}
        for t in "qkv":
            m[f"xs_{t}"] = np.ascontiguousarray(x[t][b].T)
            m[f"ws_{t}"] = ws_c[t][hg]
        in_maps.append(m)
    return in_maps


def _assemble(results):
    out = np.empty((B, L, D), np.float32)
    attn = np.empty((B, H, L, L), np.float32)
    for c in range(NCORES):
        b, hg = divmod(c, 2)
        attn[b, hg * HG : (hg + 1) * HG] = results[c]["attn"]
        out[b, :, hg * HG * DH : (hg + 1) * HG * DH] = results[c]["out"]
    return out, attn


def run(in_maps, trace=False, **kwargs):
    nc = _get_nc()
    return run_bass_kernel_spmd(
        nc, in_maps, core_ids=list(range(NCORES)), trace=trace, **kwargs
    )


def kernel(query, key, value, w_q, w_k, w_v):
    in_maps = _prep_in_maps(query, key, value, w_q, w_k, w_v)
    res = run(in_maps)
    return _assemble(res.results)


# revision 13
# speedup vs baseline: 1.5027x; 1.0075x over previous
"""Multi-head attention (B=4, L=2048, D=1024, H=16) on 8 trn2 NeuronCores.

Sharding: core c -> (batch b = c//2, head-group hg = c%2 of 8 heads).
Each core computes its batch's projections restricted to its 8 heads,
full attention for those (b, h) pairs, returning:
  attn [8, 2048, 2048] f32  and  out [2048, 512] f32
which the host reassembles into the full outputs.

v2 pipeline:
 - Projections in float32r (full PE rate, ~1.6e-4) from host-transposed
   fp32 x / w; PSUM fp32; evacuated to fp16 qT/kT (head-parity partition
   layout: even head on partitions 0-63, odd head on 64-127) and fp16 V in
   natural [l, d] layout with a ones-column per head.
 - Pass A ([k,q] layout): S.T (K=64 fp16) -> exp -> E.T fp16 -> A@V; the
   ones-column makes PSUM row 64 the softmax denominators.
 - Tiny PE transposes of [65,128] O+sums chunks -> DVE reciprocal.
 - Pass B ([q,k] layout): S -> exp -> multiply by 1/sum (per-partition
   scalar, DVE 2x) -> DMA to HBM in the natural attn layout.
"""
import numpy as np
from contextlib import ExitStack

import concourse.bass as bass
import concourse.mybir as mybir
import concourse.tile as tile
from concourse.bass import ds
from concourse.bass_utils import run_bass_kernel_spmd
from concourse.masks import make_identity

AF = mybir.ActivationFunctionType
F32 = mybir.dt.float32
F32R = mybir.dt.float32r
BF16 = mybir.dt.bfloat16
F16 = mybir.dt.float16

B, L, D, H = 4, 2048, 1024, 16
HG = 8          # heads per core
DH = 64         # head dim
P = 128
NCORES = 8
SCALE = 1.0 / 8.0   # 1/sqrt(DH)

_wait_ctr = [0]


def _split_sync_waits(nc, limit=1):
    """walrus (CoreV3) rejects instructions with too many sem waits; hoist
    excess waits onto preceding same-engine NoOps."""
    for f in nc.m.functions:
        for bb in f.blocks:
            out = []
            for inst in bb.instructions:
                si = getattr(inst, "sync_info", None)
                if si is not None and si.on_wait and len(si.on_wait) > limit:
                    waits = list(si.on_wait)
                    keep = waits[-limit:]
                    extra = waits[:-limit]
                    for i in range(0, len(extra), limit):
                        _wait_ctr[0] += 1
                        nop = mybir.InstNoOp(
                            name=f"waitsplit_nop_{_wait_ctr[0]}", ins=[], outs=[]
                        )
                        nop.engine = inst.engine
                        nop.sync_info = mybir.SyncInfo(
                            on_wait=extra[i : i + limit], on_update=[]
                        )
                        out.append(nop)
                    inst.sync_info = mybir.SyncInfo(
                        on_wait=keep, on_update=list(si.on_update)
                    )
                out.append(inst)
            bb.instructions = out
    return nc


def build_nc():
    nc = bass.Bass("TRN2", target_bir_lowering=False, debug=False)

    xs = {
        t: nc.dram_tensor(f"xs_{t}", [D, L], F32, kind="ExternalInput")
        for t in "qkv"
    }
    ws = {
        t: nc.dram_tensor(f"ws_{t}", [D, HG * DH], F32, kind="ExternalInput")
        for t in "qkv"
    }
    attn_o = nc.dram_tensor("attn", [HG, L, L], F32, kind="ExternalOutput")
    out_o = nc.dram_tensor("out", [L, HG * DH], F32, kind="ExternalOutput")

    NLB = L // 512     # 4 l-blocks
    NEO = D // P       # 8 contraction chunks
    NKC = L // P       # 16 k-chunks
    NQC = L // P       # 16 q-chunks

    with tile.TileContext(nc) as tc, ExitStack() as ctx:
        pers = ctx.enter_context(tc.tile_pool(name="pers", bufs=1))
        # per-head K-padded layout: partitions 0-63 = head dh rows, 64-127 = 0
        qs_sb = pers.tile([P, HG, L], F16, tag="qs")
        ks_sb = pers.tile([P, HG, L], F16, tag="ks")
        v_sb = pers.tile([P, NKC, HG * 65], F16, tag="v")
        ident = pers.tile([P, P], F32, tag="ident")
        make_identity(nc, ident[:])
        nc.vector.memset(v_sb[:], 1.0)  # ones-columns; data cols overwritten
        nc.vector.memset(qs_sb[ds(64, 64)], 0.0)  # zero K-pad halves
        nc.vector.memset(ks_sb[ds(64, 64)], 0.0)

        def stage_round_x(t, lb, xpool, xrp):
            """DMA an l-block of x, round to f32r."""
            x32 = xpool.tile([P, NEO, 512], F32, tag="x32")
            nc.sync.dma_start(
                x32[:],
                xs[t].ap().rearrange("(eo p) l -> p eo l", p=P)[
                    :, :, ds(lb * 512, 512)
                ],
            )
            xrt = xrp.tile([P, NEO, 512], F32R, tag="xr")
            nc.vector.tensor_copy(xrt[:], x32[:])
            return xrt

        def load_round_w(t, wpool, wstg):
            wsb = wpool.tile([P, NEO, HG * DH], F32R, tag=f"ws{t}", name=f"ws{t}")
            for wh in range(2):
                w32 = wstg.tile([P, NEO // 2, HG * DH], F32, tag="wstage")
                nc.sync.dma_start(
                    w32[:],
                    ws[t].ap().rearrange("(eo p) d -> p eo d", p=P)[
                        :, ds(wh * (NEO // 2), NEO // 2), :
                    ],
                )
                nc.vector.tensor_copy(
                    wsb[:, ds(wh * (NEO // 2), NEO // 2), :], w32[:]
                )
            return wsb

        # ---------------- Phase 1: k/q projections (float32r) ----------------
        with ExitStack() as c1:
            wpool = c1.enter_context(tc.tile_pool(name="wsb", bufs=1))
            wstg = c1.enter_context(tc.tile_pool(name="wstg", bufs=2))
            xpool = c1.enter_context(tc.tile_pool(name="xsb", bufs=2))
            xrp = c1.enter_context(tc.tile_pool(name="xrp", bufs=2))
            hstg = c1.enter_context(tc.tile_pool(name="hstg", bufs=3))
            ppsum = c1.enter_context(tc.tile_pool(name="pps", bufs=2, space="PSUM"))

            ws_sb = {t: load_round_w(t, wpool, wstg) for t in "kq"}
            for t in "kq":  # k first so attention can start during q-proj
                dst = qs_sb if t == "q" else ks_sb
                for lb in range(NLB):
                    xr = stage_round_x(t, lb, xpool, xrp)
                    for dc in range(4):
                        pst = ppsum.tile([P, 512], F32, tag="ps")
                        for eo in range(NEO):
                            nc.tensor.matmul(
                                pst[:],
                                ws_sb[t][:, eo, ds(dc * P, P)],
                                xr[:, eo, :],
                                start=(eo == 0),
                                stop=(eo == NEO - 1),
                            )
                        # even head: partitions 0-63 aligned, direct ACT copy
                        nc.scalar.copy(
                            dst[0:64, 2 * dc, ds(lb * 512, 512)], pst[0:64, :]
                        )
                        # odd head: stage fp16 then partition-shift via DMA
                        stg = hstg.tile([P, 512], F16, tag="hstg")
                        nc.scalar.copy(stg[64:128, :], pst[64:128, :])
                        nc.sync.dma_start(
                            dst[0:64, 2 * dc + 1, ds(lb * 512, 512)],
                            stg[64:128, :],
                        )

        # ---------------- Phase 2: v-projection + attention ----------------
        etp = ctx.enter_context(tc.tile_pool(name="et", bufs=10))
        e2p = ctx.enter_context(tc.tile_pool(name="e2", bufs=4))
        pstg = ctx.enter_context(tc.tile_pool(name="pst", bufs=3))
        oaugp = ctx.enter_context(tc.tile_pool(name="oaug", bufs=2))
        ostg = ctx.enter_context(tc.tile_pool(name="ostg", bufs=4))
        rp = ctx.enter_context(tc.tile_pool(name="recip", bufs=2))
        vwp = ctx.enter_context(tc.tile_pool(name="vw", bufs=1))
        vxp = ctx.enter_context(tc.tile_pool(name="vx", bufs=1))
        vxr = ctx.enter_context(tc.tile_pool(name="vxr", bufs=1))
        # st pool [128,1024] 2-bank slots; pass-B sb tiles share it
        st_ps = ctx.enter_context(tc.tile_pool(name="stp", bufs=2, space="PSUM"))
        # av pool [65,1024] 2-bank slots; v-proj psum and tr tiles share it
        av_ps = ctx.enter_context(tc.tile_pool(name="avp", bufs=2, space="PSUM"))

        # v projection (overlaps the start of attention; AV lags via et bufs)
        ws_v = load_round_w("v", vwp, vxp)
        for lb in range(NLB):
            xr = stage_round_x("v", lb, vxp, vxr)
            for lc2 in range(4):
                pst = av_ps.tile([P, 512], F32, tag="av", name="vps")
                for eo in range(NEO):
                    nc.tensor.matmul(
                        pst[:],
                        xr[:, eo, ds(lc2 * P, P)],
                        ws_v[:, eo, :],
                        start=(eo == 0),
                        stop=(eo == NEO - 1),
                    )
                lc = lb * 4 + lc2
                vv = v_sb[:, lc].rearrange("p (h x) -> p h x", h=HG)
                nc.scalar.copy(
                    vv[:, :, 0:64],
                    pst[:].rearrange("p (h d) -> p h d", h=HG),
                )

        for h in range(HG):
            rec = rp.tile([P, NQC], F32, tag="rec")
            for qbp in range(2):  # 1024 q at a time
                av = av_ps.tile([65, 1024], F32, tag="av", name="av")
                for kc in range(NKC):
                    st = st_ps.tile([P, 1024], F32, tag="st", name="st")
                    for qh in range(2):
                        q0 = qbp * 1024 + qh * 512
                        nc.tensor.matmul(
                            st[:, ds(qh * 512, 512)],
                            ks_sb[:, h, ds(kc * P, P)],
                            qs_sb[:, h, ds(q0, 512)],
                            start=True,
                            stop=True,
                        )
                    et = etp.tile([P, 1024], F16, tag="et")
                    nc.scalar.activation(et[:], st[:], AF.Exp, scale=SCALE)
                    for qh in range(2):
                        nc.tensor.matmul(
                            av[:, ds(qh * 512, 512)],
                            v_sb[:, kc, ds(h * 65, 65)],
                            et[:, ds(qh * 512, 512)],
                            start=(kc == 0),
                            stop=(kc == NKC - 1),
                        )
                oaug = oaugp.tile([65, 1024], F32, tag="oaug")
                nc.vector.tensor_copy(oaug[:], av[:])
                for half in range(2):
                    tr = av_ps.tile([P, 4, 65], F32, tag="av", name="tr")
                    for j in range(4):
                        chunk = half * 4 + j
                        nc.tensor.transpose(
                            tr[:, j, :],
                            oaug[:, ds(chunk * P, P)],
                            ident[0:65, 0:65],
                        )
                    qc0 = qbp * 8 + half * 4
                    nc.vector.reciprocal(rec[:, ds(qc0, 4)], tr[:, :, 64])
                    for j in range(4):
                        qc = qc0 + j
                        og = ostg.tile([P, DH], F32, tag="og")
                        nc.vector.tensor_scalar_mul(
                            og[:], tr[:, j, 0:DH], rec[:, ds(qc, 1)]
                        )
                        nc.sync.dma_start(
                            out_o.ap()[ds(qc * P, P), ds(h * DH, DH)], og[:]
                        )
                # pass B for this qbp's q-chunks (overlaps next kc-loop)
                for qc in range(qbp * 8, qbp * 8 + 8):
                    pstage = pstg.tile([P, L], F32, tag="pstage")
                    for kh in range(2):
                        sb = st_ps.tile([P, 1024], F32, tag="st", name="sb")
                        for kb2 in range(2):
                            nc.tensor.matmul(
                                sb[:, ds(kb2 * 512, 512)],
                                qs_sb[:, h, ds(qc * P, P)],
                                ks_sb[:, h, ds(kh * 1024 + kb2 * 512, 512)],
                                start=True,
                                stop=True,
                            )
                        e2 = e2p.tile([P, 1024], F32, tag="e2")
                        nc.scalar.activation(e2[:], sb[:], AF.Exp, scale=SCALE)
                        nc.vector.tensor_scalar_mul(
                            pstage[:, ds(kh * 1024, 1024)], e2[:], rec[:, ds(qc, 1)]
                        )
                    nc.sync.dma_start(attn_o.ap()[h, ds(qc * P, P), :], pstage[:])

    return _split_sync_waits(nc)


_NC = None


def _get_nc():
    global _NC
    if _NC is None:
        _NC = build_nc()
    return _NC


def _prep_in_maps(query, key, value, w_q, w_k, w_v):
    x = {
        "q": np.asarray(query, dtype=np.float32),
        "k": np.asarray(key, dtype=np.float32),
        "v": np.asarray(value, dtype=np.float32),
    }
    w = {
        "q": np.asarray(w_q, dtype=np.float32),
        "k": np.asarray(w_k, dtype=np.float32),
        "v": np.asarray(w_v, dtype=np.float32),
    }
    ws_c = {
        t: [
            np.ascontiguousarray(w[t][hg * HG * DH : (hg + 1) * HG * DH, :].T)
            for hg in range(2)
        ]
        for t in "qkv"
    }
    in_maps = []
    for c in range(NCORES):
        b, hg = divmod(c, 2)
        m = {}
        for t in "qkv":
            m[f"xs_{t}"] = np.ascontiguousarray(x[t][b].T)
            m[f"ws_{t}"] = ws_c[t][hg]
        in_maps.append(m)
    return in_maps


def _assemble(results):
    out = np.empty((B, L, D), np.float32)
    attn = np.empty((B, H, L, L), np.float32)
    for c in range(NCORES):
        b, hg = divmod(c, 2)
        attn[b, hg * HG : (hg + 1) * HG] = results[c]["attn"]
        out[b, :, hg * HG * DH : (hg + 1) * HG * DH] = results[c]["out"]
    return out, attn


def run(in_maps, trace=False, **kwargs):
    nc = _get_nc()
    return run_bass_kernel_spmd(
        nc, in_maps, core_ids=list(range(NCORES)), trace=trace, **kwargs
    )


def kernel(query, key, value, w_q, w_k, w_v):
    in_maps = _prep_in_maps(query, key, value, w_q, w_k, w_v)
    res = run(in_maps)
    return _assemble(res.results)


# revision 14
# speedup vs baseline: 1.5186x; 1.0106x over previous
"""Multi-head attention (B=4, L=2048, D=1024, H=16) on 8 trn2 NeuronCores.

Sharding: core c -> (batch b = c//2, head-group hg = c%2 of 8 heads).
Each core computes its batch's projections restricted to its 8 heads,
full attention for those (b, h) pairs, returning:
  attn [8, 2048, 2048] f32  and  out [2048, 512] f32
which the host reassembles into the full outputs.

v2 pipeline:
 - Projections in float32r (full PE rate, ~1.6e-4) from host-transposed
   fp32 x / w; PSUM fp32; evacuated to fp16 qT/kT (head-parity partition
   layout: even head on partitions 0-63, odd head on 64-127) and fp16 V in
   natural [l, d] layout with a ones-column per head.
 - Pass A ([k,q] layout): S.T (K=64 fp16) -> exp -> E.T fp16 -> A@V; the
   ones-column makes PSUM row 64 the softmax denominators.
 - Tiny PE transposes of [65,128] O+sums chunks -> DVE reciprocal.
 - Pass B ([q,k] layout): S -> exp -> multiply by 1/sum (per-partition
   scalar, DVE 2x) -> DMA to HBM in the natural attn layout.
"""
import numpy as np
from contextlib import ExitStack

import concourse.bass as bass
import concourse.mybir as mybir
import concourse.tile as tile
from concourse.bass import ds
from concourse.bass_utils import run_bass_kernel_spmd
from concourse.masks import make_identity

AF = mybir.ActivationFunctionType
F32 = mybir.dt.float32
F32R = mybir.dt.float32r
BF16 = mybir.dt.bfloat16
F16 = mybir.dt.float16

B, L, D, H = 4, 2048, 1024, 16
HG = 8          # heads per core
DH = 64         # head dim
P = 128
NCORES = 8
SCALE = 1.0 / 8.0   # 1/sqrt(DH)

_wait_ctr = [0]


def _split_sync_waits(nc, limit=1):
    """walrus (CoreV3) rejects instructions with too many sem waits; hoist
    excess waits onto preceding same-engine NoOps."""
    for f in nc.m.functions:
        for bb in f.blocks:
            out = []
            for inst in bb.instructions:
                si = getattr(inst, "sync_info", None)
                if si is not None and si.on_wait and len(si.on_wait) > limit:
                    waits = list(si.on_wait)
                    keep = waits[-limit:]
                    extra = waits[:-limit]
                    for i in range(0, len(extra), limit):
                        _wait_ctr[0] += 1
                        nop = mybir.InstNoOp(
                            name=f"waitsplit_nop_{_wait_ctr[0]}", ins=[], outs=[]
                        )
                        nop.engine = inst.engine
                        nop.sync_info = mybir.SyncInfo(
                            on_wait=extra[i : i + limit], on_update=[]
                        )
                        out.append(nop)
                    inst.sync_info = mybir.SyncInfo(
                        on_wait=keep, on_update=list(si.on_update)
                    )
                out.append(inst)
            bb.instructions = out
    return nc


def build_nc():
    nc = bass.Bass("TRN2", target_bir_lowering=False, debug=False)

    xs = {
        t: nc.dram_tensor(f"xs_{t}", [D, L], F32, kind="ExternalInput")
        for t in "qkv"
    }
    ws = {
        t: nc.dram_tensor(f"ws_{t}", [D, HG * DH], F32, kind="ExternalInput")
        for t in "qkv"
    }
    attn_o = nc.dram_tensor("attn", [HG, L, L], F32, kind="ExternalOutput")
    out_o = nc.dram_tensor("out", [L, HG * DH], F32, kind="ExternalOutput")

    NLB = L // 512     # 4 l-blocks
    NEO = D // P       # 8 contraction chunks
    NKC = L // P       # 16 k-chunks
    NQC = L // P       # 16 q-chunks

    with tile.TileContext(nc) as tc, ExitStack() as ctx:
        pers = ctx.enter_context(tc.tile_pool(name="pers", bufs=1))
        # fine-grained tiles for precise dependency tracking.
        # K-pad layout: partitions 0-63 = head dh rows, 64-127 = 0.
        qs_t = [
            [pers.tile([P, 1024], F16, tag=f"q{h}_{qb}", name=f"q{h}_{qb}")
             for qb in range(2)]
            for h in range(HG)
        ]
        ks_t = [
            [pers.tile([P, 512], F16, tag=f"k{h}_{lb}", name=f"k{h}_{lb}")
             for lb in range(NLB)]
            for h in range(HG)
        ]
        v_t = [
            pers.tile([P, HG * 65], F16, tag=f"v{lc}", name=f"v{lc}")
            for lc in range(NKC)
        ]
        ident = pers.tile([P, P], F32, tag="ident")
        make_identity(nc, ident[:])
        for h in range(HG):
            for qb in range(2):
                nc.vector.memset(qs_t[h][qb][ds(64, 64)], 0.0)
            for lb in range(NLB):
                nc.vector.memset(ks_t[h][lb][ds(64, 64)], 0.0)
        for lc in range(NKC):
            vv = v_t[lc].rearrange("p (h x) -> p h x", h=HG)
            nc.vector.memset(vv[:, :, 64:65], 1.0)

        # shared PSUM pools for the whole kernel (8 banks total):
        # st pool [128,1024] 2-bank slots: S~ tiles + pass-B sb tiles
        st_ps = ctx.enter_context(tc.tile_pool(name="stp", bufs=2, space="PSUM"))
        # av pool 2-bank slots: AV accumulators, all projection psum, tr tiles
        av_ps = ctx.enter_context(tc.tile_pool(name="avp", bufs=2, space="PSUM"))

        def stage_round_x(t, lb, xpool, xrp):
            x32 = xpool.tile([P, NEO, 512], F32, tag="x32")
            nc.sync.dma_start(
                x32[:],
                xs[t].ap().rearrange("(eo p) l -> p eo l", p=P)[
                    :, :, ds(lb * 512, 512)
                ],
            )
            xrt = xrp.tile([P, NEO, 512], F32R, tag="xr")
            nc.vector.tensor_copy(xrt[:], x32[:])
            return xrt

        def load_round_w(t, wpool, wstg):
            wsb = wpool.tile([P, NEO, HG * DH], F32R, tag=f"ws{t}", name=f"ws{t}")
            for wh in range(2):
                w32 = wstg.tile([P, NEO // 2, HG * DH], F32, tag="wstage")
                nc.sync.dma_start(
                    w32[:],
                    ws[t].ap().rearrange("(eo p) d -> p eo d", p=P)[
                        :, ds(wh * (NEO // 2), NEO // 2), :
                    ],
                )
                nc.vector.tensor_copy(
                    wsb[:, ds(wh * (NEO // 2), NEO // 2), :], w32[:]
                )
            return wsb

        # ---------------- Phase 1: k/q projections, interleaved ----------------
        with ExitStack() as c1:
            wpool = c1.enter_context(tc.tile_pool(name="wsb", bufs=1))
            wstg = c1.enter_context(tc.tile_pool(name="wstg", bufs=2))
            xpool = c1.enter_context(tc.tile_pool(name="xsb", bufs=2))
            xrp = c1.enter_context(tc.tile_pool(name="xrp", bufs=2))
            hstg = c1.enter_context(tc.tile_pool(name="hstg", bufs=3))

            ws_sb = {t: load_round_w(t, wpool, wstg) for t in "kq"}
            for lb in range(NLB):
                for t in "kq":
                    xr = stage_round_x(t, lb, xpool, xrp)
                    for dc in range(4):
                        pst = av_ps.tile([P, 512], F32, tag="av", name="kqps")
                        for eo in range(NEO):
                            nc.tensor.matmul(
                                pst[:],
                                ws_sb[t][:, eo, ds(dc * P, P)],
                                xr[:, eo, :],
                                start=(eo == 0),
                                stop=(eo == NEO - 1),
                            )
                        if t == "q":
                            d_ev = qs_t[2 * dc][lb // 2]
                            d_od = qs_t[2 * dc + 1][lb // 2]
                            col = ds((lb % 2) * 512, 512)
                        else:
                            d_ev = ks_t[2 * dc][lb]
                            d_od = ks_t[2 * dc + 1][lb]
                            col = ds(0, 512)
                        # even head: partitions aligned, direct ACT copy
                        nc.scalar.copy(d_ev[0:64, col], pst[0:64, :])
                        # odd head: stage fp16, partition-shift via DMA
                        stg = hstg.tile([P, 512], F16, tag="hstg")
                        nc.scalar.copy(stg[64:128, :], pst[64:128, :])
                        nc.sync.dma_start(d_od[0:64, col], stg[64:128, :])

        # ---------------- Phase 2: v-projection + attention ----------------
        etp = ctx.enter_context(tc.tile_pool(name="et", bufs=10))
        e2p = ctx.enter_context(tc.tile_pool(name="e2", bufs=4))
        pstg = ctx.enter_context(tc.tile_pool(name="pst", bufs=3))
        oaugp = ctx.enter_context(tc.tile_pool(name="oaug", bufs=2))
        ostg = ctx.enter_context(tc.tile_pool(name="ostg", bufs=4))
        rp = ctx.enter_context(tc.tile_pool(name="recip", bufs=2))
        vwp = ctx.enter_context(tc.tile_pool(name="vw", bufs=1))
        vxp = ctx.enter_context(tc.tile_pool(name="vx", bufs=1))
        vxr = ctx.enter_context(tc.tile_pool(name="vxr", bufs=1))

        ws_v = load_round_w("v", vwp, vxp)
        for lb in range(NLB):
            xr = stage_round_x("v", lb, vxp, vxr)
            for lc2 in range(4):
                pst = av_ps.tile([P, 512], F32, tag="av", name="vps")
                for eo in range(NEO):
                    nc.tensor.matmul(
                        pst[:],
                        xr[:, eo, ds(lc2 * P, P)],
                        ws_v[:, eo, :],
                        start=(eo == 0),
                        stop=(eo == NEO - 1),
                    )
                lc = lb * 4 + lc2
                vv = v_t[lc].rearrange("p (h x) -> p h x", h=HG)
                nc.vector.tensor_copy(
                    vv[:, :, 0:64],
                    pst[:].rearrange("p (h d) -> p h d", h=HG),
                )

        for h in range(HG):
            rec = rp.tile([P, NQC], F32, tag="rec")
            for qbp in range(2):  # 1024 q at a time
                av = av_ps.tile([65, 1024], F32, tag="av", name="av")
                for kc in range(NKC):
                    st = st_ps.tile([P, 1024], F32, tag="st", name="st")
                    for qh in range(2):
                        nc.tensor.matmul(
                            st[:, ds(qh * 512, 512)],
                            ks_t[h][kc // 4][:, ds((kc % 4) * P, P)],
                            qs_t[h][qbp][:, ds(qh * 512, 512)],
                            start=True,
                            stop=True,
                        )
                    et = etp.tile([P, 1024], F16, tag="et")
                    nc.scalar.activation(et[:], st[:], AF.Exp, scale=SCALE)
                    for qh in range(2):
                        nc.tensor.matmul(
                            av[:, ds(qh * 512, 512)],
                            v_t[kc][:, ds(h * 65, 65)],
                            et[:, ds(qh * 512, 512)],
                            start=(kc == 0),
                            stop=(kc == NKC - 1),
                        )
                oaug = oaugp.tile([65, 1024], F32, tag="oaug")
                nc.vector.tensor_copy(oaug[:], av[:])
                for half in range(2):
                    tr = av_ps.tile([P, 4, 65], F32, tag="av", name="tr")
                    for j in range(4):
                        chunk = half * 4 + j
                        nc.tensor.transpose(
                            tr[:, j, :],
                            oaug[:, ds(chunk * P, P)],
                            ident[0:65, 0:65],
                        )
                    qc0 = qbp * 8 + half * 4
                    nc.vector.reciprocal(rec[:, ds(qc0, 4)], tr[:, :, 64])
                    for j in range(4):
                        qc = qc0 + j
                        og = ostg.tile([P, DH], F32, tag="og")
                        nc.vector.tensor_scalar_mul(
                            og[:], tr[:, j, 0:DH], rec[:, ds(qc, 1)]
                        )
                        nc.sync.dma_start(
                            out_o.ap()[ds(qc * P, P), ds(h * DH, DH)], og[:]
                        )
                # pass B for this qbp's q-chunks (overlaps next kc-loop)
                for qc in range(qbp * 8, qbp * 8 + 8):
                    pstage = pstg.tile([P, L], F32, tag="pstage")
                    for kh in range(2):
                        sb = st_ps.tile([P, 1024], F32, tag="st", name="sb")
                        for kb2 in range(2):
                            lbk = kh * 2 + kb2
                            nc.tensor.matmul(
                                sb[:, ds(kb2 * 512, 512)],
                                qs_t[h][qbp][:, ds((qc % 8) * P, P)],
                                ks_t[h][lbk][:],
                                start=True,
                                stop=True,
                            )
                        e2 = e2p.tile([P, 1024], F32, tag="e2")
                        nc.scalar.activation(e2[:], sb[:], AF.Exp, scale=SCALE)
                        nc.vector.tensor_scalar_mul(
                            pstage[:, ds(kh * 1024, 1024)], e2[:], rec[:, ds(qc, 1)]
                        )
                    nc.sync.dma_start(attn_o.ap()[h, ds(qc * P, P), :], pstage[:])

    return _split_sync_waits(nc)


_NC = None


def _get_nc():
    global _NC
    if _NC is None:
        _NC = build_nc()
    return _NC


def _prep_in_maps(query, key, value, w_q, w_k, w_v):
    x = {
        "q": np.asarray(query, dtype=np.float32),
        "k": np.asarray(key, dtype=np.float32),
        "v": np.asarray(value, dtype=np.float32),
    }
    w = {
        "q": np.asarray(w_q, dtype=np.float32),
        "k": np.asarray(w_k, dtype=np.float32),
        "v": np.asarray(w_v, dtype=np.float32),
    }
    ws_c = {
        t: [
            np.ascontiguousarray(w[t][hg * HG * DH : (hg + 1) * HG * DH, :].T)
            for hg in range(2)
        ]
        for t in "qkv"
    }
    in_maps = []
    for c in range(NCORES):
        b, hg = divmod(c, 2)
        m = {}
        for t in "qkv":
            m[f"xs_{t}"] = np.ascontiguousarray(x[t][b].T)
            m[f"ws_{t}"] = ws_c[t][hg]
        in_maps.append(m)
    return in_maps


def _assemble(results):
    out = np.empty((B, L, D), np.float32)
    attn = np.empty((B, H, L, L), np.float32)
    for c in range(NCORES):
        b, hg = divmod(c, 2)
        attn[b, hg * HG : (hg + 1) * HG] = results[c]["attn"]
        out[b, :, hg * HG * DH : (hg + 1) * HG * DH] = results[c]["out"]
    return out, attn


def run(in_maps, trace=False, **kwargs):
    nc = _get_nc()
    return run_bass_kernel_spmd(
        nc, in_maps, core_ids=list(range(NCORES)), trace=trace, **kwargs
    )


def kernel(query, key, value, w_q, w_k, w_v):
    in_maps = _prep_in_maps(query, key, value, w_q, w_k, w_v)
    res = run(in_maps)
    return _assemble(res.results)
